# revision 5
# baseline (speedup 1.0000x reference)
"""Trainium2 Bass kernel for the Agent_LSTM_PPO problem.

Full-input contract: kernel(**inputs) takes the unsharded numpy inputs and
returns the full [3, B] output. Internally: data-parallel over batch across
8 NeuronCores (32 rows/core), LSTM recurrence on-chip in a transposed
layout (features on partitions, batch on the free dim), bf16 matmuls with
f32 accumulation, f32 cell state.

Key algebraic simplification: the reference only uses z[:, -1, :], so the
dense trunk over all timesteps is dead code — only the final LSTM hidden
state flows into the MLP heads.
"""

import os
import sys
from contextlib import ExitStack

import numpy as np

for _p in ("/opt/trn_rl_repo", "/root/.axon_site/_ro/trn_rl_repo"):
    if os.path.isdir(_p) and _p not in sys.path:
        sys.path.insert(0, _p)

import ml_dtypes  # noqa: E402

import concourse.bass as bass  # noqa: E402
import concourse.tile as tile  # noqa: E402
from concourse import bacc, mybir  # noqa: E402
from concourse.bass_utils import run_bass_kernel_spmd  # noqa: E402

BF16 = mybir.dt.bfloat16
F32 = mybir.dt.float32
NP_BF16 = ml_dtypes.bfloat16

OBS_DIM = 128
HID = 256
ACT_DIM = 32
B, S = 256, 512
NCORES = 8
BL = B // NCORES  # 32 batch rows per core

AF = mybir.ActivationFunctionType
ALU = mybir.AluOpType


def build_nc(steps=S, bl=BL):
    """Build the per-core Bass module (SPMD; same program on all cores)."""
    nc = bacc.Bacc("TRN2", target_bir_lowering=False, debug=False,
                   num_devices=NCORES)

    d = {}

    def param(name, shape, dtype):
        d[name] = nc.dram_tensor(name, shape, dtype, kind="ExternalInput")
        return d[name]

    # LSTM inputs. xT[k, t*bl + b] = obs[b, t, k]
    param("xT", [OBS_DIM, steps * bl], BF16)
    # Gate order is [f, i, o, c~]; wT[:, kc*1024 + 128*m : ...] is the
    # [128, 128] stationary block for k-chunk kc, m-chunk m.
    param("wT", [128, 3 * 4 * HID], BF16)
    param("bias_rep", [128, 8 * bl], F32)  # bias_rep[p, m*bl+b] = b[(m*128)+p]
    # Dense trunk (last timestep only)
    param("w1T", [128, 2 * 512], BF16)     # W1 [256,512] k-chunks
    param("b1T", [128, 4], F32)
    param("w2T", [128, 4 * 512], BF16)     # W2 [512,512] k-chunks
    param("b2T", [128, 4], F32)
    # Actor head
    param("a1w", [128, 4 * 64], BF16)      # A1 [512,64] k-chunks
    param("a1b", [64, 1], F32)
    param("a2w", [64, 64], BF16)
    param("a2b", [64, 1], F32)
    param("a3w", [64, ACT_DIM], BF16)
    param("a3rep", [bl, ACT_DIM], F32)     # a3 bias replicated over batch rows
    # Critic head
    param("c1w", [128, 4 * 64], BF16)
    param("c1b", [64, 1], F32)
    param("c2w", [64, 64], BF16)
    param("c2b", [64, 1], F32)
    param("c3w", [64, 1], BF16)
    param("c3rep", [bl, 1], F32)
    param("onehot", [bl, ACT_DIM], F32)

    out = nc.dram_tensor("out", [bl, 3], F32, kind="ExternalOutput")

    with tile.TileContext(nc) as tc, ExitStack() as ctx:
        const = ctx.enter_context(tc.tile_pool(name="const", bufs=1))
        state = ctx.enter_context(tc.tile_pool(name="state", bufs=1))
        work = ctx.enter_context(tc.tile_pool(name="work", bufs=3))
        head = ctx.enter_context(tc.tile_pool(name="head", bufs=1))
        ps_loop = ctx.enter_context(
            tc.tile_pool(name="ps_loop", bufs=4, space="PSUM"))
        ps_head = ctx.enter_context(
            tc.tile_pool(name="ps_head", bufs=2, space="PSUM"))

        def load(name, dtype=None):
            t = const.tile(list(d[name].shape),
                           dtype if dtype is not None else d[name].dtype,
                           tag=name)
            nc.sync.dma_start(out=t[:], in_=d[name].ap())
            return t

        xT = load("xT")
        wT = load("wT")
        bias_rep = load("bias_rep")
        w1T, b1T = load("w1T"), load("b1T")
        w2T, b2T = load("w2T"), load("b2T")
        a1w, a1b = load("a1w"), load("a1b")
        a2w, a2b = load("a2w"), load("a2b")
        a3w, a3rep = load("a3w"), load("a3rep")
        c1w, c1b = load("c1w"), load("c1b")
        c2w, c2b = load("c2w"), load("c2b")
        c3w, c3rep = load("c3w"), load("c3rep")
        onehot = load("onehot")

        hT = state.tile([128, 2 * bl], BF16)   # h.T  (k-chunk kc at cols kc*bl)
        cT = state.tile([128, 2 * bl], F32)    # c in the same layout
        nc.vector.memset(hT[:], 0.0)
        nc.vector.memset(cT[:], 0.0)

        GB = 8  # m-chunks per step (4 gates x 2 chunks of 128 features)

        for t in range(steps):
            ps = ps_loop.tile([128, GB * bl], F32, tag="ps")
            # x-part first: independent of h(t-1), PE can run ahead.
            for m in range(GB):
                nc.tensor.matmul(
                    ps[:, m * bl:(m + 1) * bl],
                    wT[:, 128 * m:128 * (m + 1)],
                    xT[:, t * bl:(t + 1) * bl],
                    start=True, stop=False)
            for m in range(GB):
                nc.tensor.matmul(
                    ps[:, m * bl:(m + 1) * bl],
                    wT[:, 1024 + 128 * m:1024 + 128 * (m + 1)],
                    hT[:, 0:bl],
                    start=False, stop=False)
                nc.tensor.matmul(
                    ps[:, m * bl:(m + 1) * bl],
                    wT[:, 2048 + 128 * m:2048 + 128 * (m + 1)],
                    hT[:, bl:2 * bl],
                    start=False, stop=True)

            gsum = work.tile([128, GB * bl], F32, tag="gsum")
            nc.vector.tensor_add(gsum[:], ps[:], bias_rep[:])
            gact = work.tile([128, GB * bl], F32, tag="gact")
            # [f,i,o] sigmoid on cols 0:6*bl, c~ tanh on cols 6*bl:8*bl
            nc.scalar.activation(gact[:, 0:6 * bl], gsum[:, 0:6 * bl],
                                 AF.Sigmoid)
            nc.scalar.activation(gact[:, 6 * bl:8 * bl],
                                 gsum[:, 6 * bl:8 * bl], AF.Tanh)

            fc = work.tile([128, 2 * bl], F32, tag="fc")
            nc.vector.tensor_mul(fc[:], gact[:, 0:2 * bl], cT[:])
            ic = work.tile([128, 2 * bl], F32, tag="ic")
            nc.vector.tensor_mul(ic[:], gact[:, 2 * bl:4 * bl],
                                 gact[:, 6 * bl:8 * bl])
            nc.vector.tensor_add(cT[:], fc[:], ic[:])
            thc = work.tile([128, 2 * bl], F32, tag="thc")
            nc.scalar.activation(thc[:], cT[:], AF.Tanh)
            nc.vector.tensor_mul(hT[:], gact[:, 4 * bl:6 * bl], thc[:])

        # ---- dense trunk on the final hidden state ----
        ps_e1 = ps_head.tile([128, 4 * bl], F32, tag="pse")
        for m in range(4):
            for kc in range(2):
                nc.tensor.matmul(
                    ps_e1[:, m * bl:(m + 1) * bl],
                    w1T[:, kc * 512 + 128 * m:kc * 512 + 128 * (m + 1)],
                    hT[:, kc * bl:(kc + 1) * bl],
                    start=(kc == 0), stop=(kc == 1))
        e1 = head.tile([128, 4 * bl], BF16)
        for m in range(4):
            nc.scalar.activation(e1[:, m * bl:(m + 1) * bl],
                                 ps_e1[:, m * bl:(m + 1) * bl],
                                 AF.Relu, bias=b1T[:, m:m + 1])

        ps_e2 = ps_head.tile([128, 4 * bl], F32, tag="pse")
        for m in range(4):
            for kc in range(4):
                nc.tensor.matmul(
                    ps_e2[:, m * bl:(m + 1) * bl],
                    w2T[:, kc * 512 + 128 * m:kc * 512 + 128 * (m + 1)],
                    e1[:, kc * bl:(kc + 1) * bl],
                    start=(kc == 0), stop=(kc == 3))
        e2 = head.tile([128, 4 * bl], BF16)
        for m in range(4):
            nc.scalar.activation(e2[:, m * bl:(m + 1) * bl],
                                 ps_e2[:, m * bl:(m + 1) * bl],
                                 AF.Relu, bias=b2T[:, m:m + 1])

        def mlp_head(pfx, w1_, b1_, w2_, b2_):
            psa = ps_head.tile([64, bl], F32, tag="psh")
            for kc in range(4):
                nc.tensor.matmul(psa[:], w1_[:, 64 * kc:64 * (kc + 1)],
                                 e2[:, kc * bl:(kc + 1) * bl],
                                 start=(kc == 0), stop=(kc == 3))
            z1 = head.tile([64, bl], BF16, tag=pfx + "_z1")
            nc.scalar.activation(z1[:], psa[:], AF.Tanh, bias=b1_[:])
            psb = ps_head.tile([64, bl], F32, tag="psh")
            nc.tensor.matmul(psb[:], w2_[:], z1[:], start=True, stop=True)
            z2 = head.tile([64, bl], BF16, tag=pfx + "_z2")
            nc.scalar.activation(z2[:], psb[:], AF.Tanh, bias=b2_[:])
            return z2

        a2t = mlp_head("a", a1w, a1b, a2w, a2b)   # [64, bl] actor features
        c2t = mlp_head("c", c1w, c1b, c2w, c2b)   # [64, bl] critic features

        # logits[b, a] : lhsT = a2t (features x batch), rhs = A3
        ps_l = ps_head.tile([bl, ACT_DIM], F32, tag="psh")
        nc.tensor.matmul(ps_l[:], a2t[:], a3w[:], start=True, stop=True)
        logits = head.tile([bl, ACT_DIM], F32)
        nc.vector.tensor_add(logits[:], ps_l[:], a3rep[:])

        ps_v = ps_head.tile([bl, 1], F32, tag="psh")
        nc.tensor.matmul(ps_v[:], c2t[:], c3w[:], start=True, stop=True)
        vals = head.tile([bl, 1], F32)
        nc.vector.tensor_add(vals[:], ps_v[:], c3rep[:])

        # ---- log-softmax, logp gather, entropy ----
        mx = head.tile([bl, 1], F32)
        nc.vector.tensor_reduce(mx[:], logits[:], axis=mybir.AxisListType.X,
                                op=ALU.max)
        negm = head.tile([bl, 1], F32)
        nc.vector.tensor_scalar_mul(negm[:], mx[:], -1.0)
        p = head.tile([bl, ACT_DIM], F32)
        nc.scalar.activation(p[:], logits[:], AF.Exp, bias=negm[:])
        ssum = head.tile([bl, 1], F32)
        nc.vector.tensor_reduce(ssum[:], p[:], axis=mybir.AxisListType.X,
                                op=ALU.add)
        logs = head.tile([bl, 1], F32)
        nc.scalar.activation(logs[:], ssum[:], AF.Ln)
        logz = head.tile([bl, 1], F32)
        nc.vector.tensor_add(logz[:], logs[:], mx[:])

        # logit_sel = sum(logits * onehot);  logp = logit_sel - logZ
        sel = head.tile([bl, ACT_DIM], F32)
        nc.vector.tensor_mul(sel[:], logits[:], onehot[:])
        lsel = head.tile([bl, 1], F32)
        nc.vector.tensor_reduce(lsel[:], sel[:], axis=mybir.AxisListType.X,
                                op=ALU.add)

        # entropy = logZ - sum(p * logits) / s
        pl = head.tile([bl, ACT_DIM], F32)
        nc.vector.tensor_mul(pl[:], p[:], logits[:])
        tsum = head.tile([bl, 1], F32)
        nc.vector.tensor_reduce(tsum[:], pl[:], axis=mybir.AxisListType.X,
                                op=ALU.add)
        rs = head.tile([bl, 1], F32)
        nc.vector.reciprocal(rs[:], ssum[:])

        outsb = head.tile([bl, 3], F32)
        # col 0: logp = lsel - logz
        nc.vector.tensor_sub(outsb[:, 0:1], lsel[:], logz[:])
        # col 1: entropy = logz - tsum*rs
        tmean = head.tile([bl, 1], F32)
        nc.vector.tensor_mul(tmean[:], tsum[:], rs[:])
        nc.vector.tensor_sub(outsb[:, 1:2], logz[:], tmean[:])
        # col 2: values
        nc.vector.tensor_copy(outsb[:, 2:3], vals[:])

        nc.sync.dma_start(out=out.ap(), in_=outsb[:])

    nc.finalize()
    return nc


def pack_inputs(obs, action, Wf, bf, Wi, bi, Wc, bc, Wo, bo,
                W1, b1, W2, b2, A1, a1, A2, a2, A3, a3,
                C1, c1, C2, c2, C3, c3, steps=S, bl=BL, ncores=NCORES):
    """Host-side sharding + layout prep. Returns list of per-core in_maps."""
    obs = np.asarray(obs, dtype=np.float32)
    action = np.asarray(action).astype(np.int64)

    # Gate order [f, i, o, c~]
    W = np.concatenate([np.asarray(Wf), np.asarray(Wi),
                        np.asarray(Wo), np.asarray(Wc)], axis=1)  # [384,1024]
    bvec = np.concatenate([np.asarray(bf), np.asarray(bi),
                           np.asarray(bo), np.asarray(bc)])       # [1024]
    wT = np.concatenate([W[k * 128:(k + 1) * 128, :] for k in range(3)],
                        axis=1).astype(NP_BF16)                    # [128,3072]
    bias_rep = np.repeat(bvec.reshape(8, 128).T[:, :, None], bl,
                         axis=2).reshape(128, 8 * bl).astype(np.float32)

    W1 = np.asarray(W1, np.float32)
    W2 = np.asarray(W2, np.float32)
    w1T = np.concatenate([W1[k * 128:(k + 1) * 128, :] for k in range(2)],
                         axis=1).astype(NP_BF16)                   # [128,1024]
    w2T = np.concatenate([W2[k * 128:(k + 1) * 128, :] for k in range(4)],
                         axis=1).astype(NP_BF16)                   # [128,2048]
    b1T = np.asarray(b1, np.float32).reshape(4, 128).T.copy()
    b2T = np.asarray(b2, np.float32).reshape(4, 128).T.copy()

    A1 = np.asarray(A1, np.float32)
    a1w = np.concatenate([A1[k * 128:(k + 1) * 128, :] for k in range(4)],
                         axis=1).astype(NP_BF16)                   # [128,256]
    C1 = np.asarray(C1, np.float32)
    c1w = np.concatenate([C1[k * 128:(k + 1) * 128, :] for k in range(4)],
                         axis=1).astype(NP_BF16)

    shared = dict(
        wT=wT, bias_rep=bias_rep, w1T=w1T, b1T=b1T, w2T=w2T, b2T=b2T,
        a1w=a1w, a1b=np.asarray(a1, np.float32).reshape(64, 1),
        a2w=np.asarray(A2, NP_BF16), a2b=np.asarray(a2, np.float32).reshape(64, 1),
        a3w=np.asarray(A3, NP_BF16),
        a3rep=np.tile(np.asarray(a3, np.float32)[None, :], (bl, 1)),
        c1w=c1w, c1b=np.asarray(c1, np.float32).reshape(64, 1),
        c2w=np.asarray(C2, NP_BF16), c2b=np.asarray(c2, np.float32).reshape(64, 1),
        c3w=np.asarray(C3, NP_BF16).reshape(64, 1),
        c3rep=np.tile(np.asarray(c3, np.float32).reshape(1, 1), (bl, 1)),
    )

    in_maps = []
    for ci in range(ncores):
        ob = obs[ci * bl:(ci + 1) * bl, :steps, :]          # [bl, steps, 128]
        xT = np.ascontiguousarray(ob.transpose(2, 1, 0)).reshape(
            OBS_DIM, steps * bl).astype(NP_BF16)
        act = action[ci * bl:(ci + 1) * bl]
        onehot = (act[:, None] == np.arange(ACT_DIM)[None, :]).astype(np.float32)
        m = dict(shared)
        m["xT"] = xT
        m["onehot"] = onehot
        in_maps.append(m)
    return in_maps


def kernel(**inputs):
    nc = build_nc()
    in_maps = pack_inputs(**inputs)
    res = run_bass_kernel_spmd(nc, in_maps, list(range(NCORES)))
    full = np.zeros((3, B), np.float32)
    for ci in range(NCORES):
        full[:, ci * BL:(ci + 1) * BL] = res.results[ci]["out"].T
    return full
